# revision 2
# baseline (speedup 1.0000x reference)
"""Grouped linear (MoE grouped GEMM) on 8 TRN2 NeuronCores via Bass/Tile.

Reference: out = ragged_dot(x, weight.swapaxes(1,2), group_lens) with
x [32768, 1024] fp32, weight [16, 1024, 1024] fp32, tokens pre-sorted
into 16 contiguous groups.

Strategy — token-parallel SPMD with host-side dispatch:
  * The host cuts each group's contiguous token run into "chunks" (one
    weight load each), each chunk into <=512-token sub-slots; an LPT
    packer balances chunks across the 8 cores.  All cores run ONE
    program whose shape is the per-position maximum profile; per-core
    numpy inputs decide which expert/tokens each position processes.
  * On-chip per sub-slot of width u: 8 out-blocks x 8 k-steps of
    [128x128] @ [128xu] bf16 matmuls accumulated in fp32 PSUM, PSUM ->
    SBUF copy (bf16), contiguous DMAs for all streams.
  * Inputs are pre-transposed/padded on the host so every DMA is
    contiguous per partition row; outputs are upcast & scattered back
    on the host.

Measured on trn2 (8 cores, seed-0 data): ~131 us/exec, rel err 3.7e-3
(bf16 compute + bf16 output quantization; fp32 accumulate).
"""

import numpy as np
import ml_dtypes

import concourse.bass as bass
import concourse.tile as tile
from concourse import bacc, mybir
from concourse.bass_utils import run_bass_kernel_spmd

G, NTOK, DIN, DOUT = 16, 32768, 1024, 1024
NCORES = 8
TT = 512           # max tokens per sub-slot
KT = DIN // 128    # 8 contraction sub-tiles
OB = DOUT // 128   # 8 output blocks
WALIGN = 16        # sub-slot width alignment (tokens)

_NC_CACHE: dict = {}


# ---------------------------------------------------------------- planner

def _split_even(total, maxpiece):
    np_ = -(-total // maxpiece)
    base = total // np_
    rem = total - base * np_
    return [base + (1 if i < rem else 0) for i in range(np_)]


def _mk_chunk(g, start, clen):
    widths = _split_even(clen, TT)
    offs = np.cumsum([0] + widths[:-1])
    return (g, [(int(start + o), int(n)) for o, n in zip(offs, widths)])


def _chunk_tok(ch):
    return sum(n for _, n in ch[1])


def _assemble(chunk_list):
    """LPT + chunk-count equalization + sorted alignment -> (profile, assign)."""
    chunks = sorted(chunk_list, key=lambda ch: -_chunk_tok(ch))
    loads = [0.0] * NCORES
    percore: list = [[] for _ in range(NCORES)]
    for ch in chunks:
        cost = sum(-(-n // WALIGN) * WALIGN for _, n in ch[1])
        c = min(range(NCORES), key=lambda i: (loads[i], len(percore[i])))
        loads[c] += cost
        percore[c].append(ch)
    maxn = max(len(pc) for pc in percore)
    for c in range(NCORES):
        while len(percore[c]) < maxn:
            percore[c].sort(key=lambda ch: -_chunk_tok(ch))
            big = percore[c][0]
            tok = _chunk_tok(big)
            if tok < 2 * WALIGN:
                break
            g = big[0]
            start = big[1][0][0]
            h1 = tok // 2
            percore[c] = [_mk_chunk(g, start, h1),
                          _mk_chunk(g, start + h1, tok - h1)] + percore[c][1:]
    for c in range(NCORES):
        percore[c].sort(key=lambda ch: (-len(ch[1]), -_chunk_tok(ch)))
    P = max(len(percore[c]) for c in range(NCORES))
    profile = []
    for p in range(P):
        m = max(len(percore[c][p][1]) if p < len(percore[c]) else 0
                for c in range(NCORES))
        widths = []
        for j in range(m):
            u = max(
                percore[c][p][1][j][1]
                if p < len(percore[c]) and j < len(percore[c][p][1])
                else 0
                for c in range(NCORES)
            )
            widths.append(-(-u // WALIGN) * WALIGN)
        profile.append(widths)
    assign = [
        [percore[c][p] if p < len(percore[c]) else None for p in range(P)]
        for c in range(NCORES)
    ]
    return profile, assign


def _plan_cost(profile):
    toks = sum(sum(w) for w in profile)
    pe_us = toks / TT * 13.6 + 6.0                    # MM stream + ramp/tail
    wmb = len(profile) * KT * DOUT * 128 * 2 / 1e6    # bf16 weights
    xmb = toks * KT * 128 * 2 / 1e6                   # bf16 activations
    omb = toks * OB * 128 * 2 / 1e6                   # bf16 outputs
    dma_us = (wmb + xmb + omb) / 0.35                 # ~350 GB/s effective
    return max(pe_us, dma_us * 1.05)


def _chunks_at_cap(group_lens, cap):
    edges = np.concatenate([[0], np.cumsum(np.asarray(group_lens, np.int64))])
    chunk_list = []
    for g in range(G):
        s, e = int(edges[g]), int(edges[g + 1])
        for clen in _split_even(e - s, cap) if e > s else []:
            chunk_list.append(_mk_chunk(g, s, clen))
            s += clen
    return chunk_list


def _plan(group_lens):
    best = None
    for cap in (4096, 3072, 2560, 2048, 1792, 1536, 1280, 1024,
                896, 768, 640, 512, 448, 384):
        profile, assign = _assemble(_chunks_at_cap(group_lens, cap))
        cost = _plan_cost(profile)
        if best is None or cost < best[0]:
            best = (cost, profile, assign)
    return best[1], best[2]


def _offsets(profile):
    xoff, ooff = [], []
    xl = ol = 0
    for widths in profile:
        xo, oo = [], []
        for u in widths:
            xo.append(xl)
            oo.append(ol)
            xl += KT * u
            ol += OB * u
        xoff.append(xo)
        ooff.append(oo)
    return xoff, ooff, xl, ol


# ------------------------------------------------------------- bass build

def _build(profile):
    key = tuple(tuple(w) for w in profile)
    if key in _NC_CACHE:
        return _NC_CACHE[key]
    dt_in = mybir.dt.bfloat16
    dt_out = mybir.dt.bfloat16
    xoff, ooff, XL, OL = _offsets(profile)
    P = len(profile)

    nc = bacc.Bacc(None, target_bir_lowering=False)
    xt = nc.declare_dram_parameter("xt", [128, XL], dt_in, isOutput=False)
    wt = nc.declare_dram_parameter("wt", [128, P * KT * DOUT], dt_in, isOutput=False)
    ot = nc.declare_dram_parameter("ot", [128, OL], dt_out, isOutput=True)

    with tile.TileContext(nc) as tc:
        with (
            tc.tile_pool(name="wp", bufs=3) as wpool,
            tc.tile_pool(name="xp", bufs=3) as xpool,
            tc.tile_pool(name="op", bufs=3) as opool,
            tc.tile_pool(name="ps", bufs=8, space=bass.MemorySpace.PSUM) as pspool,
        ):
            for p, widths in enumerate(profile):
                wsb = wpool.tile([128, KT * DOUT], dt_in, tag="wsb")
                if p == 0:
                    # split the first weight DMA per k-step so PE starts early
                    for k in range(KT):
                        nc.sync.dma_start(
                            wsb[:, k * DOUT : (k + 1) * DOUT],
                            wt[:, k * DOUT : (k + 1) * DOUT],
                        )
                else:
                    nc.sync.dma_start(
                        wsb[:, :], wt[:, p * KT * DOUT : (p + 1) * KT * DOUT]
                    )
                for j, u in enumerate(widths):
                    xsb = xpool.tile([128, KT * TT], dt_in, tag="xsb")
                    osb = opool.tile([128, OB * TT], dt_out, tag="osb")
                    if p == 0 and j == 0:
                        for k in range(KT):
                            nc.sync.dma_start(
                                xsb[:, k * u : (k + 1) * u],
                                xt[:, xoff[p][j] + k * u : xoff[p][j] + (k + 1) * u],
                            )
                    else:
                        nc.sync.dma_start(
                            xsb[:, : KT * u], xt[:, xoff[p][j] : xoff[p][j] + KT * u]
                        )
                    for o in range(OB):
                        ps = pspool.tile([128, TT], mybir.dt.float32, tag="ps")
                        for k in range(KT):
                            nc.tensor.matmul(
                                ps[:, :u],
                                wsb[:, k * DOUT + o * 128 : k * DOUT + (o + 1) * 128],
                                xsb[:, k * u : (k + 1) * u],
                                start=(k == 0),
                                stop=(k == KT - 1),
                            )
                        nc.vector.tensor_copy(osb[:, o * u : (o + 1) * u], ps[:, :u])
                    last = p == len(profile) - 1 and j == len(widths) - 1
                    if last:
                        # split the final out DMA per o-block to drain early
                        for o in range(OB):
                            nc.sync.dma_start(
                                ot[:, ooff[p][j] + o * u : ooff[p][j] + (o + 1) * u],
                                osb[:, o * u : (o + 1) * u],
                            )
                    else:
                        nc.sync.dma_start(
                            ot[:, ooff[p][j] : ooff[p][j] + OB * u], osb[:, : OB * u]
                        )

    nc.compile()
    _NC_CACHE[key] = nc
    return nc


# ----------------------------------------------------------- host scatter

def _prep_inputs(x, weight, profile, assign):
    xoff, ooff, XL, OL = _offsets(profile)
    P = len(profile)
    xbf = x.astype(ml_dtypes.bfloat16)
    # wpm[g][p, k*DOUT + o] = weight[g, o, k*128+p]
    wpm = np.ascontiguousarray(
        weight.reshape(G, DOUT, KT, 128).transpose(0, 3, 2, 1)
    ).astype(ml_dtypes.bfloat16).reshape(G, 128, KT * DOUT)
    in_maps = []
    for c in range(NCORES):
        xtc = np.zeros((128, XL), ml_dtypes.bfloat16)
        wtc = np.zeros((128, P * KT * DOUT), ml_dtypes.bfloat16)
        for p, widths in enumerate(profile):
            ch = assign[c][p]
            if ch is None:
                continue
            g, tlist = ch
            wtc[:, p * KT * DOUT : (p + 1) * KT * DOUT] = wpm[g]
            for j, (s, n) in enumerate(tlist):
                u = widths[j]
                b = np.zeros((u, DIN), ml_dtypes.bfloat16)
                b[:n] = xbf[s : s + n]
                xtc[:, xoff[p][j] : xoff[p][j] + KT * u] = (
                    b.reshape(u, KT, 128).transpose(2, 1, 0).reshape(128, KT * u)
                )
        in_maps.append({"xt": xtc, "wt": wtc})
    return in_maps


def _gather_out(results, profile, assign):
    xoff, ooff, XL, OL = _offsets(profile)
    out = np.empty((NTOK, DOUT), np.float32)
    for c in range(NCORES):
        otc = np.asarray(results[c]["ot"]).astype(np.float32)
        for p, widths in enumerate(profile):
            ch = assign[c][p]
            if ch is None:
                continue
            _, tlist = ch
            for j, (s, n) in enumerate(tlist):
                u = widths[j]
                blk = otc[:, ooff[p][j] : ooff[p][j] + OB * u].reshape(128, OB, u)
                out[s : s + n] = blk.transpose(2, 1, 0).reshape(u, DOUT)[:n]
    return out


def kernel(x, weight, group_lens):
    x = np.ascontiguousarray(np.asarray(x))
    weight = np.ascontiguousarray(np.asarray(weight))
    profile, assign = _plan(group_lens)
    nc = _build(profile)
    in_maps = _prep_inputs(x, weight, profile, assign)
    res = run_bass_kernel_spmd(nc, in_maps, list(range(NCORES)))
    return _gather_out(res.results, profile, assign)


def run_traced(x, weight, group_lens):
    """Timing/profiling entry for test harness: returns max-core exec ns."""
    x = np.ascontiguousarray(np.asarray(x))
    weight = np.ascontiguousarray(np.asarray(weight))
    profile, assign = _plan(group_lens)
    nc = _build(profile)
    in_maps = _prep_inputs(x, weight, profile, assign)
    res = run_bass_kernel_spmd(
        nc, in_maps, list(range(NCORES)), trace=True,
        trace_cores=list(range(NCORES)),
    )
    if res.instructions_and_trace is not None:
        print(f"trace: {res.instructions_and_trace[1]}")
    return res.exec_time_ns



# revision 3
# speedup vs baseline: 1.1464x; 1.1464x over previous
"""Grouped linear (MoE grouped GEMM) on 8 TRN2 NeuronCores via Bass/Tile.

Reference: out = ragged_dot(x, weight.swapaxes(1,2), group_lens) with
x [32768, 1024] fp32, weight [16, 1024, 1024] fp32, tokens pre-sorted
into 16 contiguous groups.

Strategy — token-parallel SPMD with deal-based planning:
  * Since every core runs ONE shared program, runtime depends only on
    the static profile (slot widths + weight-load count P), not on
    per-core balance.  The planner splits each group into near-equal
    pieces, sorts all pieces desc and "deals" them 8 at a time into
    rows; row width = max piece.  A local search over per-group split
    counts minimizes max(PE, DMA) cost; a tightening pass shaves
    row maxima by moving tokens between same-group pieces.
  * P ~ 7 weight loads (14.7 MB) keeps total DMA (~32 MB/core) under
    the bf16 PE stream time -> PE-bound, not DMA-bound.
  * First row: weight + x DMAs interleaved per k-step so the first
    matmul issues after ~0.4 MB of DMA.  Narrowest row goes last and
    its final out-DMA is split per o-block to shrink the drain tail.
    All DMAs stay on the single Sync HW-DGE queue: splitting streams
    across the Activation queue measured strictly worse (139 -> 146+
    us) on this workload.
  * On-chip per sub-slot of width u: 8 out-blocks x 8 k-steps of
    [128x128] @ [128xu] bf16 matmuls accumulated in fp32 PSUM, then
    PSUM -> SBUF bf16 cast, contiguous DMAs for all streams.
"""

import numpy as np
import ml_dtypes

import concourse.bass as bass
import concourse.tile as tile
from concourse import bacc, mybir
from concourse.bass_utils import run_bass_kernel_spmd

G, NTOK, DIN, DOUT = 16, 32768, 1024, 1024
NCORES = 8
TT = 512           # max tokens per sub-slot (one PSUM bank fp32)
KT = DIN // 128    # 8 contraction sub-tiles
OB = DOUT // 128   # 8 output blocks
WALIGN = 16        # row width alignment (tokens)

_NC_CACHE: dict = {}


# ---------------------------------------------------------------- planner

def _pieces_from_k(gl, kvec):
    pieces = []
    for g, k in enumerate(kvec):
        n = int(gl[g])
        if n <= 0 or k <= 0:
            continue
        base, rem = divmod(n, k)
        for i in range(k):
            pieces.append([g, base + (1 if i < rem else 0)])
    return pieces


def _deal(pieces):
    ps = sorted(pieces, key=lambda t: -t[1])
    return [ps[i: i + 8] for i in range(0, len(ps), 8)]


def _tighten(rows):
    """Shave each row's unique max piece by moving tokens to same-group
    pieces with headroom in other rows."""
    for _ in range(64):
        improved = False
        widths = [max(p[1] for p in row) for row in rows]
        for ri, row in enumerate(rows):
            w = widths[ri]
            maxps = [p for p in row if p[1] == w]
            if len(maxps) != 1:
                continue
            p = maxps[0]
            second = max((q[1] for q in row if q is not p), default=1)
            excess = p[1] - max(second, 1)
            if excess <= 0:
                continue
            for rj, row2 in enumerate(rows):
                if rj == ri or excess <= 0:
                    continue
                for q in row2:
                    if q[0] != p[0] or q is p:
                        continue
                    take = min(excess, widths[rj] - q[1])
                    if take > 0:
                        q[1] += take
                        p[1] -= take
                        excess -= take
                        improved = True
        if not improved:
            break
    return rows


def _eval_rows(rows):
    widths = [-(-max(p[1] for p in row) // WALIGN) * WALIGN for row in rows]
    sw = sum(widths)
    P = len(rows)
    pe = 26.67 * sw * 1.02
    dma = (P * 2 * 1024 * 1024 + sw * 4096) / 294.0
    return max(pe, dma) + 50.0 * P, sw, widths


def _plan(group_lens):
    gl = [int(v) for v in np.asarray(group_lens).tolist()]
    assert sum(gl) == NTOK

    best = None
    tried = set()

    def consider(kvec):
        key = tuple(kvec)
        if key in tried:
            return None
        tried.add(key)
        rows = _tighten(_deal(_pieces_from_k(gl, kvec)))
        cost, sw, widths = _eval_rows(rows)
        return (cost, rows, widths, list(kvec))

    for cap in range(256, 4128, 16):
        kvec = [-(-n // cap) if n > 0 else 0 for n in gl]
        r = consider(kvec)
        if r and (best is None or r[0] < best[0]):
            best = r

    # local search around the best kvec
    import random
    rng = random.Random(0)
    cur = best
    for _ in range(1500):
        g = rng.randrange(G)
        delta = rng.choice([-1, 1])
        kvec = cur[3][:]
        if gl[g] <= 0:
            continue
        kvec[g] += delta
        if kvec[g] < 1 or gl[g] / kvec[g] > 8 * TT:
            continue
        r = consider(kvec)
        if r and r[0] < cur[0]:
            cur = r
    best = cur

    rows, widths = best[1], best[2]
    # order rows widest-first (deal already sorted), narrowest last
    order = sorted(range(len(rows)), key=lambda i: -widths[i])
    rows = [rows[i] for i in order]
    widths = [widths[i] for i in order]

    # sub-slot widths per row: even split into ceil(w/TT) pieces, 16-aligned
    profile = []
    for w in widths:
        m = -(-w // TT)
        base = w // m
        subs = []
        acc = 0
        for j in range(m):
            u = (w - acc) // (m - j)
            u = -(-u // WALIGN) * WALIGN
            u = min(u, w - acc)
            subs.append(u)
            acc += u
        assert acc == w and all(s <= TT for s in subs)
        profile.append(subs)

    # assign pieces to cores + concrete token ranges (per-group cursor)
    cursor = {}
    edges = np.concatenate([[0], np.cumsum(gl)]).astype(np.int64)
    for g in range(G):
        cursor[g] = int(edges[g])
    assign = [[None] * len(rows) for _ in range(NCORES)]
    for p, row in enumerate(rows):
        subs = profile[p]
        for c, piece in enumerate(row):
            g, n = piece[0], int(piece[1])
            if n <= 0:
                continue
            s = cursor[g]
            cursor[g] += n
            # distribute n tokens into the row's sub-slots
            tlist = []
            rem = n
            for u in subs:
                take = min(rem, u)
                tlist.append((s, take))
                s += take
                rem -= take
            assert rem == 0
            assign[c][p] = (g, tlist)
    for g in range(G):
        assert cursor[g] == int(edges[g + 1]), (g, cursor[g])
    return profile, assign


def _offsets(profile):
    xoff, ooff = [], []
    xl = ol = 0
    for subs in profile:
        xo, oo = [], []
        for u in subs:
            xo.append(xl)
            oo.append(ol)
            xl += KT * u
            ol += OB * u
        xoff.append(xo)
        ooff.append(oo)
    return xoff, ooff, xl, ol


# ------------------------------------------------------------- bass build

def _build(profile):
    key = tuple(tuple(w) for w in profile)
    if key in _NC_CACHE:
        return _NC_CACHE[key]
    dt_in = mybir.dt.bfloat16
    dt_out = mybir.dt.bfloat16
    xoff, ooff, XL, OL = _offsets(profile)
    P = len(profile)

    nc = bacc.Bacc(None, target_bir_lowering=False)
    xt = nc.declare_dram_parameter("xt", [128, XL], dt_in, isOutput=False)
    wt = nc.declare_dram_parameter("wt", [128, P * KT * DOUT], dt_in, isOutput=False)
    ot = nc.declare_dram_parameter("ot", [128, OL], dt_out, isOutput=True)

    with tile.TileContext(nc) as tc:
        with (
            tc.tile_pool(name="wp", bufs=3) as wpool,
            tc.tile_pool(name="xp", bufs=3) as xpool,
            tc.tile_pool(name="op", bufs=3) as opool,
            tc.tile_pool(name="ps", bufs=8, space=bass.MemorySpace.PSUM) as pspool,
        ):
            for p, subs in enumerate(profile):
                wsb = wpool.tile([128, KT * DOUT], dt_in, tag="wsb")
                if p == 0:
                    # interleave w k-slices with the first x k-slices so
                    # the first matmul can issue after ~0.4 MB of DMA
                    u0 = subs[0]
                    xsb0 = xpool.tile([128, KT * TT], dt_in, tag="xsb")
                    for k in range(KT):
                        nc.sync.dma_start(
                            wsb[:, k * DOUT: (k + 1) * DOUT],
                            wt[:, k * DOUT: (k + 1) * DOUT],
                        )
                        nc.sync.dma_start(
                            xsb0[:, k * u0: (k + 1) * u0],
                            xt[:, xoff[0][0] + k * u0: xoff[0][0] + (k + 1) * u0],
                        )
                else:
                    nc.sync.dma_start(
                        wsb[:, :], wt[:, p * KT * DOUT: (p + 1) * KT * DOUT]
                    )
                for j, u in enumerate(subs):
                    if p == 0 and j == 0:
                        xsb = xsb0
                    else:
                        xsb = xpool.tile([128, KT * TT], dt_in, tag="xsb")
                        nc.sync.dma_start(
                            xsb[:, : KT * u],
                            xt[:, xoff[p][j]: xoff[p][j] + KT * u],
                        )
                    osb = opool.tile([128, OB * TT], dt_out, tag="osb")
                    for o in range(OB):
                        ps = pspool.tile([128, TT], mybir.dt.float32, tag="ps")
                        for k in range(KT):
                            nc.tensor.matmul(
                                ps[:, :u],
                                wsb[:, k * DOUT + o * 128: k * DOUT + (o + 1) * 128],
                                xsb[:, k * u: (k + 1) * u],
                                start=(k == 0),
                                stop=(k == KT - 1),
                            )
                        nc.vector.tensor_copy(osb[:, o * u: (o + 1) * u], ps[:, :u])
                    last = p == P - 1 and j == len(subs) - 1
                    if last:
                        # split the final out DMA per o-block to drain early
                        for o in range(OB):
                            nc.sync.dma_start(
                                ot[:, ooff[p][j] + o * u: ooff[p][j] + (o + 1) * u],
                                osb[:, o * u: (o + 1) * u],
                            )
                    else:
                        nc.sync.dma_start(
                            ot[:, ooff[p][j]: ooff[p][j] + OB * u],
                            osb[:, : OB * u],
                        )

    nc.compile()
    _NC_CACHE[key] = nc
    return nc


# ----------------------------------------------------------- host scatter

def _prep_inputs(x, weight, profile, assign):
    xoff, ooff, XL, OL = _offsets(profile)
    P = len(profile)
    xbf = x.astype(ml_dtypes.bfloat16)
    # wpm[g][p, k*DOUT + o] = weight[g, o, k*128+p]
    wpm = np.ascontiguousarray(
        weight.reshape(G, DOUT, KT, 128).transpose(0, 3, 2, 1)
    ).astype(ml_dtypes.bfloat16).reshape(G, 128, KT * DOUT)
    in_maps = []
    for c in range(NCORES):
        xtc = np.zeros((128, XL), ml_dtypes.bfloat16)
        wtc = np.zeros((128, P * KT * DOUT), ml_dtypes.bfloat16)
        for p, subs in enumerate(profile):
            ch = assign[c][p]
            if ch is None:
                continue
            g, tlist = ch
            wtc[:, p * KT * DOUT: (p + 1) * KT * DOUT] = wpm[g]
            for j, (s, n) in enumerate(tlist):
                u = subs[j]
                if n <= 0:
                    continue
                b = np.zeros((u, DIN), ml_dtypes.bfloat16)
                b[:n] = xbf[s: s + n]
                xtc[:, xoff[p][j]: xoff[p][j] + KT * u] = (
                    b.reshape(u, KT, 128).transpose(2, 1, 0).reshape(128, KT * u)
                )
        in_maps.append({"xt": xtc, "wt": wtc})
    return in_maps


def _gather_out(results, profile, assign):
    xoff, ooff, XL, OL = _offsets(profile)
    out = np.empty((NTOK, DOUT), np.float32)
    for c in range(NCORES):
        otc = np.asarray(results[c]["ot"]).astype(np.float32)
        for p, subs in enumerate(profile):
            ch = assign[c][p]
            if ch is None:
                continue
            _, tlist = ch
            for j, (s, n) in enumerate(tlist):
                u = subs[j]
                if n <= 0:
                    continue
                blk = otc[:, ooff[p][j]: ooff[p][j] + OB * u].reshape(128, OB, u)
                out[s: s + n] = blk.transpose(2, 1, 0).reshape(u, DOUT)[:n]
    return out


def kernel(x, weight, group_lens):
    x = np.ascontiguousarray(np.asarray(x))
    weight = np.ascontiguousarray(np.asarray(weight))
    profile, assign = _plan(group_lens)
    nc = _build(profile)
    in_maps = _prep_inputs(x, weight, profile, assign)
    res = run_bass_kernel_spmd(nc, in_maps, list(range(NCORES)))
    return _gather_out(res.results, profile, assign)


def run_traced(x, weight, group_lens):
    """Timing/profiling entry for test harness: returns max-core exec ns."""
    x = np.ascontiguousarray(np.asarray(x))
    weight = np.ascontiguousarray(np.asarray(weight))
    profile, assign = _plan(group_lens)
    nc = _build(profile)
    in_maps = _prep_inputs(x, weight, profile, assign)
    res = run_bass_kernel_spmd(
        nc, in_maps, list(range(NCORES)), trace=True,
        trace_cores=list(range(NCORES)),
    )
    if res.instructions_and_trace is not None:
        print(f"trace: {res.instructions_and_trace[1]}")
    return res.exec_time_ns
